# revision 21
# baseline (speedup 1.0000x reference)
"""Contrastive (SimCLR-style) loss on 8 Trainium2 NeuronCores.

Math (matches the reference exactly):
  P = concat(projection1, projection2)            # [8192, 256]
  sim = cos_sim(P_i, P_j); diag masked to -1e9; logits = sim / 0.5
  labels = arange(2B)  -> picks the masked diagonal, so
  loss = -mean_i( logp_ii ),  logp_ii = f32(-2e9 - lse_i),
  lse_i = log(sum_{j != i} exp(2*sim_ij))

Distribution (mirrors the data-parallel sharding hint): the 8192 rows are
sharded 1024/core.  Each core receives the "all-gathered" normalized
projection matrix Q^T as an fp8 DoubleRow-interleaved operand, plus its own
RAW row block (fp8 transposed for the matmul weights + bf16 row-major for
norms).  On chip, each core:
  - computes its block's row norms (DVE square/reduce + Newton rsqrt),
  - runs fp8 DoubleRow matmuls (K=256 per instruction) computing
    d_ij = p_i . q_j into PSUM,
  - applies exp((2/n_i) * d_ij) with the row norm folded into the
    activation's per-partition scale; row-sums come from the activation
    accumulator (ScalarE) and, for a tunable fraction of columns, from a
    Schraudolph bit-trick exp on the DVE (int16/bf16 domain) with a fused
    tensor_tensor_reduce,
  - subtracts the diagonal term and takes log via an inverse bit-trick.
Host applies the reference's fp32 arithmetic for the final mean.
"""

import sys

for _p in ("/opt/trn_rl_repo", "/root/.axon_site/_ro/trn_rl_repo"):
    if _p not in sys.path:
        sys.path.append(_p)

import numpy as np
import ml_dtypes

import concourse.bacc as bacc
import concourse.tile as tile
from concourse import mybir
from concourse import bass_utils

F32 = mybir.dt.float32
BF16 = mybir.dt.bfloat16
FP8 = mybir.dt.float8e4
I16 = mybir.dt.int16
I32 = mybir.dt.int32
AF = mybir.ActivationFunctionType
ALU = mybir.AluOpType
PERF = mybir.MatmulPerfMode

N_CORES = 8
B = 8192          # total rows (2 * batch)
D = 256           # projection dim
BLK = B // N_CORES        # 1024 rows per core
M_TILES = BLK // 128      # 8 row tiles per core
N_GROUPS = 4              # column groups of 2048
GROUP = B // N_GROUPS     # 2048

# Schraudolph exp/log constants (f32 bit domain)
A_EXP = 12102203.161561485      # 2^23 / ln 2
B_EXP = 1064866805.0            # 127*2^23 - mean-centering offset
A16 = A_EXP / 65536.0           # int16/bf16-domain variants
B16 = B_EXP / 65536.0
E2 = 7.38905609893065           # exp(2): diagonal term to subtract
RSQRT_MAGIC = 0x5F3759DF

# Per-(m,g) cell split of the 2048 columns into ScalarE/DVE chunks.
# 'A' chunks go through the exp activation (<=1536 cols, 3 PSUM banks),
# 'D' chunks (512 cols, 1 bank) go through the DVE bit-exp path.
P1 = (("A", 1536), ("D", 512))
P2 = (("A", 1024), ("D", 512), ("D", 512))
P3 = (("D", 512),) * 4


ACT_ONLY = False


def _cell_patterns():
    grid = {}
    for m in range(M_TILES):
        for g in range(N_GROUPS):
            if ACT_ONLY:
                grid[(m, g)] = (("A", 1024), ("A", 1024))
            elif m == 1 and g in (0, 2):
                grid[(m, g)] = P3
            else:
                grid[(m, g)] = P1
    return grid


CELLS = _cell_patterns()


def _newton_rsqrt(nc, pool, out_rn, s, final_scale=1.0):
    """out_rn = final_scale/sqrt(s), entirely on VectorE (fp32)."""
    p, w = s.shape
    ibits = pool.tile([p, w], I32, name="ibits", tag="rsq_i", bufs=1)
    nc.vector.tensor_scalar(
        out=ibits, in0=s.bitcast(I32), scalar1=1, scalar2=None,
        op0=ALU.arith_shift_right,
    )
    nc.vector.tensor_scalar(
        out=ibits, in0=ibits, scalar1=-1, scalar2=RSQRT_MAGIC,
        op0=ALU.mult, op1=ALU.add,
    )
    y = ibits.bitcast(F32)
    t1 = pool.tile([p, w], F32, name="t1", tag="rsq_t1", bufs=1)
    for it in range(2):
        nc.vector.tensor_mul(t1, y, y)
        nc.vector.tensor_mul(t1, t1, s)
        nc.vector.tensor_scalar(
            out=t1, in0=t1, scalar1=-0.5, scalar2=1.5,
            op0=ALU.mult, op1=ALU.add,
        )
        if it == 0:
            nc.vector.tensor_mul(y, y, t1)
        else:
            nc.vector.tensor_mul(t1, y, t1)
    # t1 holds 1/sqrt(s)
    nc.vector.tensor_scalar_mul(out_rn, t1, float(final_scale))


def _emit(tc, qt8, wt8, pb, lse_out):
    nc = tc.nc

    persist = tc.alloc_tile_pool(name="persist", bufs=1)
    work = tc.alloc_tile_pool(name="work", bufs=2)
    epool = tc.alloc_tile_pool(name="epool", bufs=2)

    # ---- SBUF persistent tensors ----
    qt_s = persist.tile([128, 2, B], FP8, tag="qt_s", name="qt_s")
    wt_s = persist.tile([128, 2, BLK], FP8, tag="wt_s", name="wt_s")
    pb_s = persist.tile([128, M_TILES, D], BF16, tag="pb_s", name="pb_s")
    rn2 = persist.tile([128, M_TILES], F32, tag="rn2", name="rn2")
    cexp = persist.tile([128, M_TILES], F32, tag="cexp", name="cexp")
    sums = persist.tile([128, M_TILES, 16], F32, tag="sums", name="sums")
    lse = persist.tile([128, M_TILES], F32, tag="lse", name="lse")

    # ---- DMA loads: sync carries the matmul operands (order = priority);
    # pb rides the gpsimd queue in parallel so the norm prologue starts
    # early.  The first 1536 columns load separately so the first ACT
    # PSUM slot fills as soon as possible. ----
    nc.sync.dma_start(out=wt_s, in_=wt8)
    nc.gpsimd.dma_start(out=pb_s, in_=pb)
    nc.sync.dma_start(out=qt_s[:, :, 0:1536], in_=qt8[:, :, 0:1536])
    nc.sync.dma_start(out=qt_s[:, :, 1536:GROUP], in_=qt8[:, :, 1536:GROUP])
    for g in range(1, N_GROUPS):
        nc.sync.dma_start(
            out=qt_s[:, :, g * GROUP : (g + 1) * GROUP],
            in_=qt8[:, :, g * GROUP : (g + 1) * GROUP],
        )

    nc.vector.memset(sums, 0.0)

    # ---- Prologue: own-block row norms -> activation scales.
    # m=0 runs as a tiny fast chain so the first ACT chunk isn't gated
    # on the full prologue; m=1..7 run batched. ----
    sq = work.tile([128, M_TILES, D], BF16, name="sq", tag="sq", bufs=1)
    nsq = work.tile([128, M_TILES], F32, name="nsq", tag="nsq", bufs=1)
    for sl in (slice(0, 1), slice(1, M_TILES)):
        nc.vector.tensor_mul(sq[:, sl, :], pb_s[:, sl, :], pb_s[:, sl, :])
        nc.vector.tensor_reduce(
            nsq[:, sl], sq[:, sl, :], axis=mybir.AxisListType.X, op=ALU.add
        )
        # rn2 = 2/n_i (activation scale), cexp = (2*A_EXP)/n_i (bit-exp)
        _newton_rsqrt(nc, work, rn2[:, sl], nsq[:, sl], final_scale=2.0)
        nc.vector.tensor_scalar_mul(cexp[:, sl], rn2[:, sl], float(A_EXP))

    # ---- PSUM pools: ScalarE slots (2x3 banks) + DVE slots (2x1 bank) ----
    ps_act = tc.alloc_tile_pool(name="ps_act", bufs=2, space="PSUM")
    ps_dve = tc.alloc_tile_pool(name="ps_dve", bufs=2, space="PSUM")

    # ---- Main loop: m-outer (weights stay stationary), g-inner ----
    for m in range(M_TILES):
        wslice = wt_s[:, :, m * 128 : (m + 1) * 128]
        slot = [0]         # running partial-sum slot index (max 16 per m)
        pend = [None, 0]   # current paired i32 bit-exp buffer, fill count

        def flush_dve(force=False):
            eb2, nfill = pend
            if eb2 is None:
                return
            if nfill == 2:
                nc.vector.tensor_reduce(
                    sums[:, m, slot[0] : slot[0] + 2],
                    eb2.bitcast(F32),
                    axis=mybir.AxisListType.X,
                    op=ALU.add,
                )
                slot[0] += 2
                pend[0], pend[1] = None, 0
            elif force and nfill == 1:
                nc.vector.tensor_reduce(
                    sums[:, m, slot[0] : slot[0] + 1],
                    eb2[:, 0, :].bitcast(F32),
                    axis=mybir.AxisListType.X,
                    op=ALU.add,
                )
                slot[0] += 1
                pend[0], pend[1] = None, 0

        for g in range(N_GROUPS):
            col0 = g * GROUP
            off = 0
            for kind, ncols in CELLS[(m, g)]:
                if kind == "A":
                    ps = ps_act.tile([128, 1536], F32, name="ps_a")
                    for n4 in range(ncols // 512):
                        c = col0 + off + n4 * 512
                        nc.tensor.matmul(
                            ps[:, n4 * 512 : (n4 + 1) * 512],
                            wslice,
                            qt_s[:, :, c : c + 512],
                            start=True, stop=True,
                            perf_mode=PERF.DoubleRow,
                        )
                    esc = epool.tile([128, 1536], BF16, name="esc", tag="esc",
                                     bufs=2)
                    nc.scalar.activation(
                        out=esc[:, :ncols],
                        in_=ps[:, :ncols],
                        func=AF.Exp,
                        scale=rn2[:, m : m + 1],
                        accum_out=sums[:, m, slot[0] : slot[0] + 1],
                    )
                    slot[0] += 1
                else:
                    ps = ps_dve.tile([128, 512], F32, name="ps_d")
                    c = col0 + off
                    nc.tensor.matmul(
                        ps, wslice, qt_s[:, :, c : c + 512],
                        start=True, stop=True,
                        perf_mode=PERF.DoubleRow,
                    )
                    if pend[0] is None:
                        pend[0] = epool.tile([128, 2, 512], I32, name="eb",
                                             tag="eb", bufs=3)
                        pend[1] = 0
                    nc.vector.tensor_scalar(
                        out=pend[0][:, pend[1], :], in0=ps,
                        scalar1=cexp[:, m : m + 1], scalar2=float(B_EXP),
                        op0=ALU.mult, op1=ALU.add,
                    )
                    pend[1] += 1
                    flush_dve()
                off += ncols
        flush_dve(force=True)

    # ---- Epilogue ----
    rowsum = persist.tile([128, M_TILES], F32, tag="rowsum", name="rowsum")
    nc.vector.tensor_reduce(rowsum, sums, axis=mybir.AxisListType.X, op=ALU.add)
    nc.vector.tensor_scalar_add(rowsum, rowsum, -float(E2))
    # bit-trick ln: lse = (bits(x) - B_EXP)/A_EXP
    nc.vector.tensor_scalar(
        out=lse, in0=rowsum.bitcast(I32),
        scalar1=1.0 / A_EXP, scalar2=-B_EXP / A_EXP,
        op0=ALU.mult, op1=ALU.add,
    )
    nc.sync.dma_start(out=lse_out, in_=lse)

    for p in (epool, ps_dve, ps_act, work, persist):
        p.release()


_BUILT = None


def _build():
    global _BUILT
    if _BUILT is None:
        nc = bacc.Bacc("TRN2", target_bir_lowering=False, debug=False,
                       num_devices=N_CORES)
        qt8 = nc.dram_tensor("qt8", [128, 2, B], FP8, kind="ExternalInput").ap()
        wt8 = nc.dram_tensor("wt8", [128, 2, BLK], FP8,
                             kind="ExternalInput").ap()
        pb = nc.dram_tensor("pb", [128, M_TILES, D], BF16,
                            kind="ExternalInput").ap()
        lse_out = nc.dram_tensor("lse_out", [128, M_TILES], F32,
                                 kind="ExternalOutput").ap()
        with tile.TileContext(nc) as tc:
            _emit(tc, qt8, wt8, pb, lse_out)
        nc.finalize()
        _BUILT = nc
    return _BUILT


def _host_prep(P):
    """Host-side staging: normalized fp8 Q^T (DoubleRow-interleaved), raw
    fp8 block weights, bf16 row-major blocks for on-chip norms."""
    n = np.linalg.norm(P, axis=1, keepdims=True)
    Q = P / n
    # qt8[ki, ko, j] = Q[j, 128*ko + ki]
    qt8 = np.clip(Q.T, -240, 240).reshape(2, 128, B).transpose(1, 0, 2)
    qt8 = np.ascontiguousarray(qt8).astype(ml_dtypes.float8_e4m3)
    wt8s, pbs = [], []
    for c in range(N_CORES):
        Pb = P[c * BLK : (c + 1) * BLK]
        wt8 = np.clip(Pb.T, -240, 240).reshape(2, 128, BLK).transpose(1, 0, 2)
        wt8s.append(np.ascontiguousarray(wt8).astype(ml_dtypes.float8_e4m3))
        pb = Pb.reshape(M_TILES, 128, D).transpose(1, 0, 2)
        pbs.append(np.ascontiguousarray(pb).astype(ml_dtypes.bfloat16))
    return qt8, wt8s, pbs


def run_on_hw(P, **spmd_kwargs):
    nc = _build()
    qt8, wt8s, pbs = _host_prep(P)
    in_maps = [
        {"qt8": qt8, "wt8": wt8s[c], "pb": pbs[c]} for c in range(N_CORES)
    ]
    return bass_utils.run_bass_kernel_spmd(
        nc, in_maps, core_ids=list(range(N_CORES)), **spmd_kwargs
    )


DEBUG_LSE = None


def kernel(embedding1, embedding2, projection1, projection2):
    import jax.numpy as jnp

    global DEBUG_LSE
    # embeddings are unused by the reference computation
    P = np.ascontiguousarray(
        np.concatenate([projection1, projection2], axis=0), dtype=np.float32
    )
    res = run_on_hw(P)
    # reassemble per-row lse: core c, tile column t, partition p ->
    # global row c*1024 + t*128 + p
    lse_rows = np.empty(B, np.float32)
    for c in range(N_CORES):
        arr = np.asarray(res.results[c]["lse_out"])  # [128, M_TILES]
        lse_rows[c * BLK : (c + 1) * BLK] = arr.T.reshape(-1)
    DEBUG_LSE = lse_rows
    # Reference fp32 semantics: logp_ii = f32(-2e9 - lse_i) (== -2e9 for
    # any |lse| < 128), then loss = -mean(logp) with the platform's XLA
    # fp32 reduction -- reproduce it bit-for-bit.
    logp = (np.float32(-2.0e9) - lse_rows).astype(np.float32)
    loss = -jnp.mean(jnp.asarray(logp))
    return np.asarray(loss)


# revision 22
# speedup vs baseline: 1.0237x; 1.0237x over previous
"""Contrastive (SimCLR-style) loss on 8 Trainium2 NeuronCores.

Math (matches the reference exactly):
  P = concat(projection1, projection2)            # [8192, 256]
  sim = cos_sim(P_i, P_j); diag masked to -1e9; logits = sim / 0.5
  labels = arange(2B)  -> picks the masked diagonal, so
  loss = -mean_i( logp_ii ),  logp_ii = f32(-2e9 - lse_i),
  lse_i = log(sum_{j != i} exp(2*sim_ij))

Distribution (mirrors the data-parallel sharding hint): the 8192 rows are
sharded 1024/core.  Each core receives the "all-gathered" normalized
projection matrix Q^T as an fp8 DoubleRow-interleaved operand, plus its own
RAW row block (fp8 transposed for the matmul weights + bf16 row-major for
norms).  On chip, each core:
  - computes its block's row norms (DVE square/reduce + Newton rsqrt),
  - runs fp8 DoubleRow matmuls (K=256 per instruction) computing
    d_ij = p_i . q_j into PSUM,
  - applies exp((2/n_i) * d_ij) with the row norm folded into the
    activation's per-partition scale; row-sums come from the activation
    accumulator (ScalarE) and, for a tunable fraction of columns, from a
    Schraudolph bit-trick exp on the DVE (int16/bf16 domain) with a fused
    tensor_tensor_reduce,
  - subtracts the diagonal term and takes log via an inverse bit-trick.
Host applies the reference's fp32 arithmetic for the final mean.
"""

import sys

for _p in ("/opt/trn_rl_repo", "/root/.axon_site/_ro/trn_rl_repo"):
    if _p not in sys.path:
        sys.path.append(_p)

import numpy as np
import ml_dtypes

import concourse.bacc as bacc
import concourse.tile as tile
from concourse import mybir
from concourse import bass_utils

F32 = mybir.dt.float32
BF16 = mybir.dt.bfloat16
FP8 = mybir.dt.float8e4
I16 = mybir.dt.int16
I32 = mybir.dt.int32
AF = mybir.ActivationFunctionType
ALU = mybir.AluOpType
PERF = mybir.MatmulPerfMode

N_CORES = 8
B = 8192          # total rows (2 * batch)
D = 256           # projection dim
BLK = B // N_CORES        # 1024 rows per core
M_TILES = BLK // 128      # 8 row tiles per core
N_GROUPS = 4              # column groups of 2048
GROUP = B // N_GROUPS     # 2048

# Schraudolph exp/log constants (f32 bit domain)
A_EXP = 12102203.161561485      # 2^23 / ln 2
B_EXP = 1064866805.0            # 127*2^23 - mean-centering offset
A16 = A_EXP / 65536.0           # int16/bf16-domain variants
B16 = B_EXP / 65536.0
E2 = 7.38905609893065           # exp(2): diagonal term to subtract
RSQRT_MAGIC = 0x5F3759DF

# Per-(m,g) cell split of the 2048 columns into ScalarE/DVE chunks.
# 'A' chunks go through the exp activation (<=1536 cols, 3 PSUM banks),
# 'D' chunks (512 cols, 1 bank) go through the DVE bit-exp path.
P1 = (("A", 1536), ("D", 512))
P2 = (("A", 1024), ("D", 512), ("D", 512))
P3 = (("D", 512),) * 4


ACT_ONLY = False


def _cell_patterns():
    grid = {}
    for m in range(M_TILES):
        for g in range(N_GROUPS):
            if ACT_ONLY:
                grid[(m, g)] = (("A", 1024), ("A", 1024))
            elif m == 1 and g in (0, 2):
                grid[(m, g)] = P3
            else:
                grid[(m, g)] = P1
    return grid


CELLS = _cell_patterns()


def _newton_rsqrt(nc, pool, out_rn, s, final_scale=1.0):
    """out_rn = final_scale/sqrt(s), entirely on VectorE (fp32)."""
    p, w = s.shape
    ibits = pool.tile([p, w], I32, name="ibits", tag="rsq_i", bufs=1)
    nc.vector.tensor_scalar(
        out=ibits, in0=s.bitcast(I32), scalar1=1, scalar2=None,
        op0=ALU.arith_shift_right,
    )
    nc.vector.tensor_scalar(
        out=ibits, in0=ibits, scalar1=-1, scalar2=RSQRT_MAGIC,
        op0=ALU.mult, op1=ALU.add,
    )
    y = ibits.bitcast(F32)
    t1 = pool.tile([p, w], F32, name="t1", tag="rsq_t1", bufs=1)
    for it in range(2):
        nc.vector.tensor_mul(t1, y, y)
        nc.vector.tensor_mul(t1, t1, s)
        nc.vector.tensor_scalar(
            out=t1, in0=t1, scalar1=-0.5, scalar2=1.5,
            op0=ALU.mult, op1=ALU.add,
        )
        if it == 0:
            nc.vector.tensor_mul(y, y, t1)
        else:
            nc.vector.tensor_mul(t1, y, t1)
    # t1 holds 1/sqrt(s)
    nc.vector.tensor_scalar_mul(out_rn, t1, float(final_scale))


def _emit(tc, qt8, wt8, pb, lse_out):
    nc = tc.nc

    persist = tc.alloc_tile_pool(name="persist", bufs=1)
    work = tc.alloc_tile_pool(name="work", bufs=2)
    epool = tc.alloc_tile_pool(name="epool", bufs=2)

    # ---- SBUF persistent tensors ----
    qt_s = persist.tile([128, 2, B], FP8, tag="qt_s", name="qt_s")
    wt_s = persist.tile([128, 2, BLK], FP8, tag="wt_s", name="wt_s")
    pb_s = persist.tile([128, M_TILES, D], BF16, tag="pb_s", name="pb_s")
    rn2 = persist.tile([128, M_TILES], F32, tag="rn2", name="rn2")
    cexp = persist.tile([128, M_TILES], F32, tag="cexp", name="cexp")
    sums = persist.tile([128, M_TILES, 16], F32, tag="sums", name="sums")
    lse = persist.tile([128, M_TILES], F32, tag="lse", name="lse")

    # Dummy activation with no data deps: forces the exp ACT_TABLE_LOAD
    # to run during the DMA window instead of blocking the first real chunk.
    warm = persist.tile([128, 1], F32, tag="warm", name="warm")
    nc.vector.memset(warm, 0.0)
    nc.scalar.activation(out=warm, in_=warm, func=AF.Exp)

    # ---- DMA loads: sync carries the matmul operands (order = priority);
    # pb rides the gpsimd queue in parallel so the norm prologue starts
    # early.  The first 1536 columns load separately so the first ACT
    # PSUM slot fills as soon as possible. ----
    nc.sync.dma_start(out=wt_s, in_=wt8)
    nc.gpsimd.dma_start(out=pb_s, in_=pb)
    nc.sync.dma_start(out=qt_s[:, :, 0:1536], in_=qt8[:, :, 0:1536])
    nc.sync.dma_start(out=qt_s[:, :, 1536:GROUP], in_=qt8[:, :, 1536:GROUP])
    for g in range(1, N_GROUPS):
        nc.sync.dma_start(
            out=qt_s[:, :, g * GROUP : (g + 1) * GROUP],
            in_=qt8[:, :, g * GROUP : (g + 1) * GROUP],
        )

    nc.vector.memset(sums, 0.0)

    # ---- Prologue: own-block row norms -> activation scales.
    # m=0 runs as a tiny fast chain so the first ACT chunk isn't gated
    # on the full prologue; m=1..7 run batched. ----
    sq = work.tile([128, M_TILES, D], BF16, name="sq", tag="sq", bufs=1)
    nsq = work.tile([128, M_TILES], F32, name="nsq", tag="nsq", bufs=1)
    for sl in (slice(0, 1), slice(1, M_TILES)):
        nc.vector.tensor_mul(sq[:, sl, :], pb_s[:, sl, :], pb_s[:, sl, :])
        nc.vector.tensor_reduce(
            nsq[:, sl], sq[:, sl, :], axis=mybir.AxisListType.X, op=ALU.add
        )
        # rn2 = 2/n_i (activation scale), cexp = (2*A_EXP)/n_i (bit-exp)
        _newton_rsqrt(nc, work, rn2[:, sl], nsq[:, sl], final_scale=2.0)
        nc.vector.tensor_scalar_mul(cexp[:, sl], rn2[:, sl], float(A_EXP))

    # ---- PSUM pools: ScalarE slots (2x3 banks) + DVE slots (2x1 bank) ----
    ps_act = tc.alloc_tile_pool(name="ps_act", bufs=2, space="PSUM")
    ps_dve = tc.alloc_tile_pool(name="ps_dve", bufs=2, space="PSUM")

    # ---- Main loop: m-outer (weights stay stationary), g-inner ----
    for m in range(M_TILES):
        wslice = wt_s[:, :, m * 128 : (m + 1) * 128]
        slot = [0]         # running partial-sum slot index (max 16 per m)
        pend = [None, 0]   # current paired i32 bit-exp buffer, fill count

        def flush_dve(force=False):
            eb2, nfill = pend
            if eb2 is None:
                return
            if nfill == 2:
                nc.vector.tensor_reduce(
                    sums[:, m, slot[0] : slot[0] + 2],
                    eb2.bitcast(F32),
                    axis=mybir.AxisListType.X,
                    op=ALU.add,
                )
                slot[0] += 2
                pend[0], pend[1] = None, 0
            elif force and nfill == 1:
                nc.vector.tensor_reduce(
                    sums[:, m, slot[0] : slot[0] + 1],
                    eb2[:, 0, :].bitcast(F32),
                    axis=mybir.AxisListType.X,
                    op=ALU.add,
                )
                slot[0] += 1
                pend[0], pend[1] = None, 0

        for g in range(N_GROUPS):
            col0 = g * GROUP
            off = 0
            for kind, ncols in CELLS[(m, g)]:
                if kind == "A":
                    ps = ps_act.tile([128, 1536], F32, name="ps_a")
                    for n4 in range(ncols // 512):
                        c = col0 + off + n4 * 512
                        nc.tensor.matmul(
                            ps[:, n4 * 512 : (n4 + 1) * 512],
                            wslice,
                            qt_s[:, :, c : c + 512],
                            start=True, stop=True,
                            perf_mode=PERF.DoubleRow,
                        )
                    esc = epool.tile([128, 1536], BF16, name="esc", tag="esc",
                                     bufs=2)
                    nc.scalar.activation(
                        out=esc[:, :ncols],
                        in_=ps[:, :ncols],
                        func=AF.Exp,
                        scale=rn2[:, m : m + 1],
                        accum_out=sums[:, m, slot[0] : slot[0] + 1],
                    )
                    slot[0] += 1
                else:
                    ps = ps_dve.tile([128, 512], F32, name="ps_d")
                    c = col0 + off
                    nc.tensor.matmul(
                        ps, wslice, qt_s[:, :, c : c + 512],
                        start=True, stop=True,
                        perf_mode=PERF.DoubleRow,
                    )
                    if pend[0] is None:
                        pend[0] = epool.tile([128, 2, 512], I32, name="eb",
                                             tag="eb", bufs=3)
                        pend[1] = 0
                    nc.vector.tensor_scalar(
                        out=pend[0][:, pend[1], :], in0=ps,
                        scalar1=cexp[:, m : m + 1], scalar2=float(B_EXP),
                        op0=ALU.mult, op1=ALU.add,
                    )
                    pend[1] += 1
                    flush_dve()
                off += ncols
        flush_dve(force=True)

    # ---- Epilogue ----
    rowsum = persist.tile([128, M_TILES], F32, tag="rowsum", name="rowsum")
    nc.vector.tensor_reduce(rowsum, sums, axis=mybir.AxisListType.X, op=ALU.add)
    nc.vector.tensor_scalar_add(rowsum, rowsum, -float(E2))
    # bit-trick ln: lse = (bits(x) - B_EXP)/A_EXP
    nc.vector.tensor_scalar(
        out=lse, in0=rowsum.bitcast(I32),
        scalar1=1.0 / A_EXP, scalar2=-B_EXP / A_EXP,
        op0=ALU.mult, op1=ALU.add,
    )
    nc.sync.dma_start(out=lse_out, in_=lse)

    for p in (epool, ps_dve, ps_act, work, persist):
        p.release()


_BUILT = None


def _build():
    global _BUILT
    if _BUILT is None:
        nc = bacc.Bacc("TRN2", target_bir_lowering=False, debug=False,
                       num_devices=N_CORES)
        qt8 = nc.dram_tensor("qt8", [128, 2, B], FP8, kind="ExternalInput").ap()
        wt8 = nc.dram_tensor("wt8", [128, 2, BLK], FP8,
                             kind="ExternalInput").ap()
        pb = nc.dram_tensor("pb", [128, M_TILES, D], BF16,
                            kind="ExternalInput").ap()
        lse_out = nc.dram_tensor("lse_out", [128, M_TILES], F32,
                                 kind="ExternalOutput").ap()
        with tile.TileContext(nc) as tc:
            _emit(tc, qt8, wt8, pb, lse_out)
        nc.finalize()
        _BUILT = nc
    return _BUILT


def _host_prep(P):
    """Host-side staging: normalized fp8 Q^T (DoubleRow-interleaved), raw
    fp8 block weights, bf16 row-major blocks for on-chip norms."""
    n = np.linalg.norm(P, axis=1, keepdims=True)
    Q = P / n
    # qt8[ki, ko, j] = Q[j, 128*ko + ki]
    qt8 = np.clip(Q.T, -240, 240).reshape(2, 128, B).transpose(1, 0, 2)
    qt8 = np.ascontiguousarray(qt8).astype(ml_dtypes.float8_e4m3)
    wt8s, pbs = [], []
    for c in range(N_CORES):
        Pb = P[c * BLK : (c + 1) * BLK]
        wt8 = np.clip(Pb.T, -240, 240).reshape(2, 128, BLK).transpose(1, 0, 2)
        wt8s.append(np.ascontiguousarray(wt8).astype(ml_dtypes.float8_e4m3))
        pb = Pb.reshape(M_TILES, 128, D).transpose(1, 0, 2)
        pbs.append(np.ascontiguousarray(pb).astype(ml_dtypes.bfloat16))
    return qt8, wt8s, pbs


def run_on_hw(P, **spmd_kwargs):
    nc = _build()
    qt8, wt8s, pbs = _host_prep(P)
    in_maps = [
        {"qt8": qt8, "wt8": wt8s[c], "pb": pbs[c]} for c in range(N_CORES)
    ]
    return bass_utils.run_bass_kernel_spmd(
        nc, in_maps, core_ids=list(range(N_CORES)), **spmd_kwargs
    )


DEBUG_LSE = None


def kernel(embedding1, embedding2, projection1, projection2):
    import jax.numpy as jnp

    global DEBUG_LSE
    # embeddings are unused by the reference computation
    P = np.ascontiguousarray(
        np.concatenate([projection1, projection2], axis=0), dtype=np.float32
    )
    res = run_on_hw(P)
    # reassemble per-row lse: core c, tile column t, partition p ->
    # global row c*1024 + t*128 + p
    lse_rows = np.empty(B, np.float32)
    for c in range(N_CORES):
        arr = np.asarray(res.results[c]["lse_out"])  # [128, M_TILES]
        lse_rows[c * BLK : (c + 1) * BLK] = arr.T.reshape(-1)
    DEBUG_LSE = lse_rows
    # Reference fp32 semantics: logp_ii = f32(-2e9 - lse_i) (== -2e9 for
    # any |lse| < 128), then loss = -mean(logp) with the platform's XLA
    # fp32 reduction -- reproduce it bit-for-bit.
    logp = (np.float32(-2.0e9) - lse_rows).astype(np.float32)
    loss = -jnp.mean(jnp.asarray(logp))
    return np.asarray(loss)
